# revision 56
# baseline (speedup 1.0000x reference)
"""Trainium2 distributed kernel for nn_AttentionLayer (dense cross-attention
with sink-competition softmax) — v2.

Sharding: 8 cores = 2 batches x 4 head-pairs.  Core c handles batch c//4 and
heads {2*(c%4), 2*(c%4)+1}.

v2 structural changes vs v1:
  - Host ships activations pre-transposed ([feat, tokens], bf16, token-tile
    blocked) so the kernel does zero PE transposes of activations and half
    the HBM traffic.
  - LayerNorm folded into the projections:  LN(x)@W = rstd*(x@W' - c X mu) + b
    with c = gamma@W.  Sums/sums-of-squares come from ones-matmuls against
    the transposed activations (sliding-ones lhsT accumulates all token
    tiles into one [10, 512] PSUM block); squares computed on the otherwise
    idle GpSimd engine; rstd = exp(-0.5*ln(var+eps)) keeps ScalarE in the
    ln/exp table sets only.
  - Key-side rstd is folded into the Exp's per-partition scale AP, query-side
    rstd applied via a rank-1 broadcast, so normalized q/k are never
    materialized separately.
  - sim matmuls for the two heads are row-packed (K=64 at array rows 0-63 /
    64-127) so they run concurrently in the PE array.
  - Output collective is an AllGather of the per-core [128, 1024] bf16
    attention output (instead of ReduceScatter of [1024, 512] f32); each
    core then computes its 256 output rows against the full Wo.
"""

import contextlib
import os
import sys

for _p in ("/opt/trn_rl_repo", "/root/.axon_site/_ro/trn_rl_repo"):
    if os.path.isdir(_p) and _p not in sys.path:
        sys.path.insert(0, _p)

import numpy as np
import ml_dtypes

# Defensive: concourse.bass_utils imports antenv.axon_hooks on the traced
# path; provide a no-op registry if the environment lacks it so tracing
# degrades instead of crashing.
try:
    import antenv.axon_hooks  # noqa: F401
except Exception:
    import types as _types

    _ah = _types.ModuleType("antenv.axon_hooks")
    _ah._hook = None
    _ah.set_axon_ntff_profile_hook = lambda h: setattr(_ah, "_hook", h)
    _ah.get_axon_ntff_profile_hook = lambda: getattr(_ah, "_hook", None)
    try:
        import antenv as _antenv
        _antenv.axon_hooks = _ah
    except Exception:
        pass
    sys.modules["antenv.axon_hooks"] = _ah

import concourse.bass as bass
import concourse.bacc as bacc
import concourse.mybir as mybir
import concourse.tile as tile
from concourse.bass_utils import run_bass_kernel_spmd

# Pin Exp/Ln (the only table-backed activations we use) to the one set that
# contains both, so the table loader never thrashes between the exp-only and
# ln-only sets (each switch costs ~2.7us of ScalarE time).
_ORIG_GAT = bacc.get_activation_tables


def _gat_single_set(arch):
    tabs = dict(_ORIG_GAT(arch))
    _AF = mybir.ActivationFunctionType
    pinned = {_AF.Exp, _AF.Ln, _AF.Square}
    if any(pinned <= fns for fns in tabs.values()):
        out = {}
        seen = False
        for name, fns in tabs.items():
            if not seen and pinned <= fns:
                out[name] = fns
                seen = True
            else:
                out[name] = fns - pinned
        return out
    return tabs


bacc.get_activation_tables = _gat_single_set

F32 = mybir.dt.float32
BF16 = mybir.dt.bfloat16
FP8 = mybir.dt.float8e4
DR = mybir.MatmulPerfMode.DoubleRow
AF = mybir.ActivationFunctionType
ALU = mybir.AluOpType
S_LT = 4096.0  # scale lifting lt = v/Z into fp8e4 range (cancels in av/C)

B, N_SINK, N_SRC, DIM, HID, H = 2, 1024, 4096, 512, 512, 8
D_HEAD = HID // H            # 64
EPS = 1e-6
SCALE = D_HEAD ** -0.5       # 0.125
N_CORES = 8
GROUP = 4                    # cores per batch group
N_TT = 10                    # token tiles of 512 (2 sink + 8 source)
N_KT = 32                    # key tiles of 128
OUT_ROWS = N_SINK // GROUP   # 256 output rows per core

LAST_RESULT = None


def _stat_row(jt):
    """token-tile index for source key-tile jt (tokens 128*jt..+128)."""
    return 2 + jt // 4


def _col_idx(jt):
    """column in the rstd-cols tiles for key-tile jt (8-row source block)."""
    return 8 * (jt % 4) + jt // 4


def build_bass(has_bias=True):
    nc = bacc.Bacc(None, target_bir_lowering=False, debug=False,
                   num_devices=N_CORES)

    for val in (EPS, SCALE, -0.5, 0.5):
        t = nc.alloc_sbuf_tensor(f"const-f32-{val}", [128, 1], F32)
        nc.gpsimd.memset(t.ap(), val)
        nc.const_aps.aps[(F32, val)] = t.ap()
    nc.all_engine_barrier()

    # ---- per-core DRAM parameters ----
    sinkT_d = nc.declare_dram_parameter("sinkT", [128, 2 * 2048], BF16, isOutput=False)
    srcT_d = nc.declare_dram_parameter("srcT", [128, 8 * 2048], BF16, isOutput=False)
    wq_d = nc.declare_dram_parameter("wq", [128, 512], BF16, isOutput=False)
    wk_d = nc.declare_dram_parameter("wk", [128, 512], BF16, isOutput=False)
    wv_d = nc.declare_dram_parameter("wv", [128, 512], BF16, isOutput=False)
    wo_d = nc.declare_dram_parameter("wo", [128, 2048], BF16, isOutput=False)
    r1q_d = nc.declare_dram_parameter("r1q", [1, 128], BF16, isOutput=False)
    r1k_d = nc.declare_dram_parameter("r1k", [1, 128], BF16, isOutput=False)
    r1v_d = nc.declare_dram_parameter("r1v", [1, 128], BF16, isOutput=False)
    bo4_d = nc.declare_dram_parameter("bo4", [1, 512], BF16, isOutput=False)
    ident_d = nc.declare_dram_parameter("ident", [128, 128], BF16, isOutput=False)
    slide_d = nc.declare_dram_parameter("slide", [128, 32], BF16, isOutput=False)
    onec_d = nc.declare_dram_parameter("ones_c", [128, 1], BF16, isOutput=False)
    oner_d = nc.declare_dram_parameter("ones_r", [1, 512], BF16, isOutput=False)
    bo_d = nc.declare_dram_parameter("bo_r", [1, 512], BF16, isOutput=False)
    out_d = nc.declare_dram_parameter("out", [N_SINK, DIM], F32, isOutput=True)

    # collective bounce buffers: per-head 4-rank (batch-group) AllGather of
    # the per-core attn-out half [64 hid, 1024 q].  Output rows 64s = group
    # rank s's head-h dims; every core then computes the full batch output
    # and the host keeps one copy per batch.  Head 0's gather overlaps head
    # 1's key loop.
    ag_in = [nc.dram_tensor(f"ag_in{h}", [64, 1024], BF16) for h in range(2)]
    ag_out = [nc.dram_tensor(f"ag_out{h}", [256, 1024], BF16)
              for h in range(2)]
    # bounce row for the 1/C broadcast (DRAM source legally broadcasts)
    rc_d = [nc.dram_tensor(f"rc_d{h}", [1, 1024], BF16) for h in range(2)]
    # bounce rows for the sink-rstd broadcast onto qT
    rq_d = [nc.dram_tensor(f"rq_d{g}", [1, 512], BF16) for g in range(2)]
    # dummy collective to absorb the ~11us first-collective wakeup latency
    warm_in = nc.dram_tensor("warm_in", [1, 64], BF16)
    warm_out = nc.dram_tensor("warm_out", [4, 64], BF16)
    # stats block -> flat-row bounce buffers ([10, 512] -> [1, 5120]),
    # needed only for the rank-1 bias matmuls
    flat_names = ("ir",) if has_bias else ()
    flat_d = {nm: nc.dram_tensor(f"{nm}_flat_d", [1, 10 * 512], BF16)
              for nm in flat_names}

    with tile.TileContext(nc) as tc:
        with tc.tile_pool(name="const", bufs=1) as cp:
            wq = cp.tile([128, 512], BF16, name="wq_sb")
            wk = cp.tile([128, 512], BF16, name="wk_sb")
            wv = cp.tile([128, 512], BF16, name="wv_sb")
            wo = cp.tile([128, 2048], BF16, name="wo_sb")
            r1q = cp.tile([1, 128], BF16, name="r1q_sb")
            r1k = cp.tile([1, 128], BF16, name="r1k_sb")
            r1v = cp.tile([1, 128], BF16, name="r1v_sb")
            bo4 = cp.tile([1, 512], BF16, name="bo4_sb")
            ident = cp.tile([128, 128], BF16, name="id_sb")
            slide = cp.tile([128, 32], BF16, name="slide_sb")
            ones_c = cp.tile([128, 1], BF16, name="ones_c_sb")
            ones_r = cp.tile([1, 512], BF16, name="ones_r_sb")
            bo_r = cp.tile([1, 512], BF16, name="bo_r_sb")
            # activations, blocked [128, 2048*t + 512*cc + tau]; issue the
            # tensors the stats pipeline needs first.
            xT = cp.tile([128, 20 * 1024], BF16, name="xT_sb")
            # Spread input DMAs over four DGE rings: activations split across
            # sync/scalar/vector, small weights on sync behind sinkT, and
            # late-use tensors (wo, biases) on gpsimd's ring.
            nc.sync.dma_start(out=slide[:, :], in_=slide_d[:, :])
            nc.sync.dma_start(out=xT[:, 0:4096], in_=sinkT_d[:, :])
            for piece, eng in enumerate((nc.scalar, nc.sync,
                                         nc.scalar, nc.sync)):
                eng.dma_start(
                    out=xT[:, 4096 * (piece + 1):4096 * (piece + 2)],
                    in_=srcT_d[:, 4096 * piece:4096 * (piece + 1)])
            for sb, dr in ((wq, wq_d), (wk, wk_d), (wv, wv_d),
                           (ident, ident_d)):
                nc.sync.dma_start(out=sb[:, :], in_=dr[:, :])
            for sb, dr in ((wo, wo_d), (r1q, r1q_d), (r1k, r1k_d),
                           (r1v, r1v_d), (bo4, bo4_d), (ones_c, onec_d),
                           (ones_r, oner_d), (bo_r, bo_d)):
                nc.scalar.dma_start(out=sb[:, :], in_=dr[:, :])

            # persistent activations / stats
            qT = cp.tile([128, 1024], BF16, name="qT_sb")
            kT = cp.tile([128, 4096], BF16, name="kT_sb")
            v_sb = cp.tile([128, 130 * N_KT], BF16, name="v_sb")
            ir_bf = cp.tile([10, 512], BF16, name="ir_bf")
            rstd_s = cp.tile([2, 512], BF16, name="rstd_s")
            rstd_c = cp.tile([8, 512], BF16, name="rstd_c")
            # flat-row copy (base partition 0) for rank-1 bias operands
            ir_fl = cp.tile([1, 5120], BF16, name="ir_flat")
            scol = cp.tile([128, 40], F32, name="scol_sb")
            rcol = cp.tile([128, 40], F32, name="rcol_sb")
            aoT = cp.tile([128, 1024], BF16, name="aoT_sb")
            ao_s = cp.tile([128, 4 * 1024], BF16, name="ao_s_sb")

            def xslc(t, cc, off, width):
                return xT[:, 2048 * t + 512 * cc + off:
                          2048 * t + 512 * cc + off + width]

            # ------------- stats (sink block first, then source) -------------
            # v_sb data cols are overwritten with v*rstd*S_LT; the memset
            # value survives only in the C columns (64/129 of each block),
            # which must carry the same S_LT scale so it cancels in av/C.
            nc.gpsimd.memset(v_sb[:, :], S_LT)
            # projection pools outlive both the stats and attention regions
            outer = contextlib.ExitStack()
            pjp = outer.enter_context(
                tc.tile_pool(name="pj_ps", bufs=2, space="PSUM"))
            bsb = outer.enter_context(tc.tile_pool(name="bc_sb", bufs=2))
            with tc.tile_pool(name="xsq", bufs=6) as sqp, \
                 tc.tile_pool(name="st_ps", bufs=1, space="PSUM") as stp, \
                 tc.tile_pool(name="st_sb", bufs=2) as ssb:
                sqs = {}

                def square(t, cc):
                    # keep ScalarE free for the rstd chain + exp stream
                    xs = xslc(t, cc, 0, 512)
                    sq = sqp.tile([128, 512], BF16, tag="sq", bufs=40,
                                  name=f"sq{t}_{cc}")
                    if (4 * t + cc) % 4 < 3:
                        nc.vector.tensor_tensor(sq[:, :], xs, xs, ALU.mult)
                    else:
                        nc.gpsimd.tensor_tensor(sq[:, :], xs, xs, ALU.mult)
                    sqs[(t, cc)] = sq

                def stat_block(t0, rows, rstd_out, irows):
                    """sums + rstd for token tiles t0..t0+rows-1."""
                    sx = stp.tile([rows, 512], F32, tag=f"sxa{t0}", bufs=1,
                                  name=f"sx_{t0}")
                    sx2 = stp.tile([rows, 512], F32, tag=f"sxb{t0}", bufs=1,
                                   name=f"sx2_{t0}")
                    for t in range(t0, t0 + rows):
                        lh = slide[:, 10 - (t - t0):10 - (t - t0) + rows]
                        for cc in range(4):
                            nc.tensor.matmul(sx[0:rows, :], lh,
                                             xslc(t, cc, 0, 512),
                                             start=(t == t0 and cc == 0),
                                             stop=(t == t0 + rows - 1
                                                   and cc == 3))
                    for t in range(t0, t0 + rows):
                        lh = slide[:, 10 - (t - t0):10 - (t - t0) + rows]
                        for cc in range(4):
                            nc.tensor.matmul(sx2[0:rows, :], lh,
                                             sqs[(t, cc)][:, :],
                                             start=(t == t0 and cc == 0),
                                             stop=(t == t0 + rows - 1
                                                   and cc == 3))
                    mu = ssb.tile([rows, 512], F32, tag=f"mu{t0}", bufs=1,
                                  name=f"mu_{t0}")
                    var = ssb.tile([rows, 512], F32, tag=f"va{t0}", bufs=1,
                                   name=f"var_{t0}")
                    lnv = ssb.tile([rows, 512], F32, tag=f"ln{t0}", bufs=1,
                                   name=f"lnv_{t0}")
                    nc.vector.tensor_scalar(mu[:, :], sx[:, :], 1.0 / DIM,
                                            None, ALU.mult)
                    nc.vector.tensor_scalar(var[:, :], sx2[:, :], 1.0 / DIM,
                                            None, ALU.mult)
                    nc.vector.tensor_tensor(mu[:, :], mu[:, :], mu[:, :],
                                            ALU.mult)
                    nc.vector.tensor_tensor(var[:, :], var[:, :], mu[:, :],
                                            ALU.subtract)
                    nc.scalar.activation(lnv[:, :], var[:, :], AF.Ln,
                                         bias=EPS)
                    nc.scalar.activation(rstd_out[:, :], lnv[:, :], AF.Exp,
                                         scale=-0.5)
                    if has_bias:
                        nc.scalar.activation(ir_bf[irows, :], lnv[:, :],
                                             AF.Exp, scale=0.5)

                def q_proj():
                    for g in range(2):
                        bcs = bsb.tile([128, 512], BF16, tag="bcs",
                                       name=f"bcs{g}")
                        nc.sync.dma_start(out=rq_d[g].ap(),
                                          in_=rstd_s[g:g + 1, :])
                        nc.sync.dma_start(
                            out=bcs[:, :],
                            in_=rq_d[g].ap().partition_broadcast(128))
                        pj = pjp.tile([128, 512], F32, tag="pj",
                                      name=f"pjq{g}")
                        for cc in range(4):
                            nc.tensor.matmul(pj[:, :],
                                             wq[:, 128 * cc:128 * (cc + 1)],
                                             xslc(g, cc, 0, 512),
                                             start=(cc == 0),
                                             stop=(cc == 3 and not has_bias))
                        if has_bias:
                            nc.tensor.matmul(pj[:, :], r1q[0:1, :],
                                             ir_fl[0:1, 512 * g:512 * (g + 1)],
                                             start=False, stop=True)
                        nc.vector.tensor_tensor(qT[:, 512 * g:512 * (g + 1)],
                                                pj[:, :], bcs[:, :], ALU.mult)

                def k_proj(g):
                    t = 2 + g
                    pj = pjp.tile([128, 512], F32, tag="pj", name=f"pjk{g}")
                    for cc in range(4):
                        nc.tensor.matmul(pj[:, :],
                                         wk[:, 128 * cc:128 * (cc + 1)],
                                         xslc(t, cc, 0, 512),
                                         start=(cc == 0),
                                         stop=(cc == 3 and not has_bias))
                    if has_bias:
                        nc.tensor.matmul(pj[:, :], r1k[0:1, :],
                                         ir_fl[0:1, 512 * t:512 * (t + 1)],
                                         start=False, stop=True)
                    nc.vector.tensor_copy(kT[:, 512 * g:512 * (g + 1)],
                                          pj[:, :])

                def v_proj(jt):
                    t, w = _stat_row(jt), 128 * (jt % 4)
                    vp = pjp.tile([128, 512], F32, tag="pj", name=f"vp{jt}")
                    for cc in range(4):
                        nc.tensor.matmul(vp[:, 0:128], xslc(t, cc, w, 128),
                                         wv[:, 128 * cc:128 * (cc + 1)],
                                         start=(cc == 0),
                                         stop=(cc == 3 and not has_bias))
                    if has_bias:
                        nc.tensor.matmul(
                            vp[:, 0:128],
                            ir_fl[0:1, 512 * t + w:512 * t + w + 128],
                            r1v[0:1, :], start=False, stop=True)
                    ci = _col_idx(jt)
                    vb = v_sb[:, 130 * jt:130 * jt + 130]
                    nc.vector.tensor_scalar(vb[:, 0:64], vp[:, 0:64],
                                            rcol[:, ci:ci + 1],
                                            None, ALU.mult)
                    nc.vector.tensor_scalar(vb[:, 65:129], vp[:, 64:128],
                                            rcol[:, ci:ci + 1],
                                            None, ALU.mult)

                for t in range(2):
                    for cc in range(4):
                        square(t, cc)
                stat_block(0, 2, rstd_s, slice(0, 2))
                # the first projection band runs on the PE while the source
                # tiles are still arriving
                q_proj()
                k_proj(0)
                for t in range(2, N_TT):
                    for cc in range(4):
                        square(t, cc)
                stat_block(2, 8, rstd_c, slice(2, 10))
                if has_bias:
                    dview = flat_d["ir"].ap().rearrange(
                        "a (t n) -> (a t) n", t=10)
                    nc.sync.dma_start(out=dview, in_=ir_bf[:, :])
                    nc.sync.dma_start(out=ir_fl[:, :],
                                      in_=flat_d["ir"].ap())

                # rstd columns for the source tiles (exp scale + v scaling)
                with tc.tile_pool(name="tp_ps", bufs=2, space="PSUM") as tpp:
                    for c4 in range(4):
                        tp = tpp.tile([128, 8], BF16, tag="tp", name=f"tp{c4}")
                        nc.tensor.transpose(
                            tp[:, :], rstd_c[0:8, 128 * c4:128 * (c4 + 1)],
                            ident[0:8, 0:8])
                        nc.vector.tensor_scalar(
                            scol[:, 8 * c4:8 * (c4 + 1)], tp[:, :], SCALE,
                            None, ALU.mult)
                        nc.vector.tensor_scalar(
                            rcol[:, 8 * c4:8 * (c4 + 1)], tp[:, :], S_LT,
                            None, ALU.mult)
                # first v band, now that rcol exists
                for j in range(4):
                    v_proj(j)

            # ------------- head-split attention, projections interleaved -------------
            with tc.tile_pool(name="acc_ps", bufs=1, space="PSUM") as accp, \
                 tc.tile_pool(name="sim_ps", bufs=2, space="PSUM") as simp, \
                 tc.tile_pool(name="att", bufs=4) as ap_, \
                 tc.tile_pool(name="rs", bufs=4) as rsp, \
                 tc.tile_pool(name="ep_sb", bufs=2) as epp:

                # warm the CC stream well before AG1 (no input deps; issued
                # here so its SDMA traffic stays clear of the input loads)
                nc.gpsimd.collective_compute(
                    "AllGather", ALU.bypass,
                    replica_groups=[[0, 1, 2, 3], [4, 5, 6, 7]],
                    ins=[warm_in.ap().opt()],
                    outs=[warm_out.ap().opt()],
                )
                for h in range(2):
                    hs = 64 * h
                    acc = accp.tile([65, 1024], F32, tag="acc",
                                    name=f"acc{h}")
                    exs = {}
                    lts = {}

                    def issue_sim(jt, h=h, hs=hs, exs=exs, lts=lts):
                        """sim matmul + exp + lt build for key tile jt."""
                        ci = _col_idx(jt)
                        s2 = rsp.tile([128, 1], F32, tag="s2",
                                      name=f"s2_{h}_{jt}")
                        sim = simp.tile([128, 1024], F32, tag="sim",
                                        name=f"sim{h}_{jt}")
                        for qc in range(2):
                            nc.tensor.matmul(
                                sim[:, 512 * qc:512 * (qc + 1)],
                                kT[hs:hs + 64, 128 * jt:128 * (jt + 1)],
                                qT[hs:hs + 64, 512 * qc:512 * (qc + 1)],
                                start=True, stop=True)
                        ex = ap_.tile([128, 1024], BF16, tag="ex",
                                      bufs=6, name=f"ex{h}_{jt}")
                        nc.scalar.activation(ex[:, :], sim[:, :],
                                             AF.Exp,
                                             scale=scol[:, ci:ci + 1],
                                             accum_out=s2[:, 0:1])
                        exs[jt] = ex
                        rs2 = rsp.tile([128, 1], F32, tag="rs2",
                                       name=f"rs2_{h}_{jt}")
                        nc.vector.reciprocal(rs2[:, :], s2[:, :])
                        lt = ap_.tile([128, 65], BF16, tag="lt", bufs=3,
                                      name=f"lt{h}_{jt}")
                        vb = v_sb[:, 130 * jt + 65 * h:
                                  130 * jt + 65 * (h + 1)]
                        nc.vector.tensor_scalar(lt[:, :], vb,
                                                rs2[:, 0:1], None, ALU.mult)
                        lts[jt] = lt

                    def issue_av(jt, acc=acc, exs=exs, lts=lts):
                        lt = lts.pop(jt)
                        ex = exs.pop(jt)
                        for qc in range(2):
                            nc.tensor.matmul(
                                acc[0:65, 512 * qc:512 * (qc + 1)],
                                lt[:, :],
                                ex[:, 512 * qc:512 * (qc + 1)],
                                start=(jt == 0), stop=(jt == N_KT - 1))

                    if h == 0:
                        # interleave the remaining k/v projections with head
                        # 0's key loop: band g's projections land just before
                        # the sims that consume them (band 0 was issued
                        # during the stats region).
                        for g in range(8):
                            if g > 0:
                                k_proj(g)
                                for j in range(4):
                                    v_proj(4 * g + j)
                            for jt in range(4 * g, 4 * g + 4):
                                issue_sim(jt)
                                if jt > 0:
                                    issue_av(jt - 1)
                        issue_av(N_KT - 1)
                    else:
                        issue_sim(0)
                        for jt in range(N_KT):
                            if jt + 1 < N_KT:
                                issue_sim(jt + 1)
                            issue_av(jt)

                    # head epilogue: 1/C = exp(-ln C), DMA-broadcast to the
                    # 64 row partitions, normalize, ship to the collective.
                    lnC = epp.tile([1, 1024], F32, tag="lnC", name=f"lnC{h}")
                    nc.scalar.activation(lnC[:, :], acc[64:65, :], AF.Ln)
                    rcb = epp.tile([1, 1024], BF16, tag="rcb", name=f"rcb{h}")
                    nc.scalar.activation(rcb[:, :], lnC[:, :], AF.Exp,
                                         scale=-1.0)
                    bcs = epp.tile([64, 1024], BF16, tag="bcs", name=f"bcs{h}")
                    nc.sync.dma_start(out=rc_d[h].ap(), in_=rcb[0:1, :])
                    nc.sync.dma_start(
                        out=bcs[:, :],
                        in_=rc_d[h].ap().partition_broadcast(64))
                    nc.vector.tensor_tensor(aoT[hs:hs + 64, :], acc[0:64, :],
                                            bcs[:, :], ALU.mult)
                    nc.sync.dma_start(out=ag_in[h].ap(),
                                      in_=aoT[hs:hs + 64, :])
                    nc.gpsimd.collective_compute(
                        "AllGather", ALU.bypass,
                        replica_groups=[[0, 1, 2, 3], [4, 5, 6, 7]],
                        ins=[ag_in[h].ap().opt()],
                        outs=[ag_out[h].ap().opt()],
                    )

            outer.close()

            # ---- readback + full-batch final projection ----
            # ao_s partitions 0-63 = even-head dims, 64-127 = odd-head dims;
            # columns are [q-chunk, rank, q].  The even-head half of every
            # final accumulation runs while head 1's AllGather drains.
            for qc in range(8):
                for h in range(2):
                    eng = nc.sync if qc % 2 == 0 else nc.scalar
                    eng.dma_start(
                        out=ao_s[64 * h:64 * (h + 1),
                                 512 * qc:512 * (qc + 1)].rearrange(
                            "p (s n) -> p s n", s=4),
                        in_=ag_out[h].ap()[:, 128 * qc:128 * (qc + 1)]
                        .rearrange("(s p) n -> p s n", s=4))

            with tc.tile_pool(name="f_ps", bufs=8, space="PSUM") as fpp, \
                 tc.tile_pool(name="fout", bufs=4) as fop:
                fs = []
                # even-head half of every accumulation first: it only needs
                # AG1, so it runs while head 1's AllGather drains.
                for qc in range(8):
                    f = fpp.tile([128, 512], F32, tag="f", name=f"f{qc}")
                    fs.append(f)
                    for blk in range(4):
                        nc.tensor.matmul(
                            f[:, :],
                            ao_s[0:64, 512 * qc + 128 * blk:
                                 512 * qc + 128 * (blk + 1)],
                            wo[0:64, 512 * blk:512 * (blk + 1)],
                            start=(blk == 0), stop=False)
                for qc in range(8):
                    f = fs[qc]
                    for blk in range(4):
                        nc.tensor.matmul(
                            f[:, :],
                            ao_s[64:128, 512 * qc + 128 * blk:
                                 512 * qc + 128 * (blk + 1)],
                            wo[64:128, 512 * blk:512 * (blk + 1)],
                            start=False,
                            stop=(blk == 3 and not has_bias))
                    if has_bias:
                        nc.tensor.matmul(f[:, :], ones_r[0:1, 0:128],
                                         bo_r[0:1, :], start=False, stop=True)
                    fo = fop.tile([128, 512], F32, tag="fo", name=f"fo{qc}")
                    nc.vector.tensor_copy(fo[:, :], f[:, :])
                    nc.sync.dma_start(out=out_d[128 * qc:128 * (qc + 1), :],
                                      in_=fo[:, :])

    return nc


def _blk(xT):
    """[512, T] f32 -> [128, 4*T] bf16, col = 2048*t + 512*cc + tau."""
    T = xT.shape[1]
    nt = T // 512
    out = xT.reshape(4, 128, nt, 512).transpose(1, 2, 0, 3).reshape(128, 4 * T)
    return np.ascontiguousarray(out).astype(ml_dtypes.bfloat16)


def _chunked(w_loc):
    """[512, 128] -> [128, 512] with col = 128*cc + d."""
    return np.ascontiguousarray(
        w_loc.reshape(4, 128, 128).transpose(1, 0, 2).reshape(128, 512))


def make_in_maps(sink, source, gamma_s, beta_s, gamma_c, beta_c,
                 Wq, bq, Wkv, bkv, Wo, bo):
    f32 = np.float32
    bf16 = ml_dtypes.bfloat16
    cq = (gamma_s @ Wq).astype(f32)
    ck = (gamma_c @ Wkv[:, :HID]).astype(f32)
    cv = (gamma_c @ Wkv[:, HID:]).astype(f32)
    # LN fold: rstd_i*(x_i @ W_eff) + b_eff == LN(x_i) @ (gamma*W) + b, with
    # the mean correction folded into the weights as a rank-1 update.
    Wq_eff = (Wq * gamma_s[:, None] - cq[None, :] / DIM).astype(f32)
    bq_eff = (bq + beta_s @ Wq).astype(f32)
    Wkv_eff = (Wkv * gamma_c[:, None]
               - np.concatenate([ck, cv])[None, :] / DIM).astype(f32)
    bkv_eff = (bkv + beta_c @ Wkv).astype(f32)
    Wk_f, Wv_f = Wkv_eff[:, :HID], Wkv_eff[:, HID:]
    bk_f, bv_f = bkv_eff[:HID], bkv_eff[HID:]

    ident = np.eye(128, dtype=f32).astype(bf16)
    slide = np.zeros((128, 32), f32)
    slide[:, 10] = 1.0
    slide = slide.astype(bf16)
    ones_c = np.ones((128, 1), f32).astype(bf16)
    ones_r = np.ones((1, 512), f32).astype(bf16)
    bo_r = bo.reshape(1, 512).astype(bf16)
    bo4 = (bo / GROUP).reshape(1, 512).astype(bf16)

    # [128, 4*512]: block s = Wo rows for group-rank s's hid slice.
    wo_grp = np.concatenate(
        [Wo[128 * s:128 * (s + 1), :] for s in range(GROUP)], axis=1)

    in_maps = []
    for c in range(N_CORES):
        b, hp = c // GROUP, c % GROUP
        cols = slice(128 * hp, 128 * hp + 128)
        in_maps.append({
            "sinkT": _blk(np.ascontiguousarray(sink[b].T).astype(f32)),
            "srcT": _blk(np.ascontiguousarray(source[b].T).astype(f32)),
            "wq": _chunked(Wq_eff[:, cols]).astype(bf16),
            "wk": _chunked(Wk_f[:, cols]).astype(bf16),
            "wv": _chunked(Wv_f[:, cols]).astype(bf16),
            "wo": wo_grp.astype(bf16),
            "r1q": bq_eff[cols][None, :].astype(bf16),
            "r1k": bk_f[cols][None, :].astype(bf16),
            "r1v": bv_f[cols][None, :].astype(bf16),
            "bo4": bo4,
            "ident": ident,
            "slide": slide,
            "ones_c": ones_c,
            "ones_r": ones_r,
            "bo_r": bo_r,
        })
    return in_maps


_NC_CACHE = {}


def kernel(**inputs):
    global LAST_RESULT
    has_bias = bool(
        np.any(inputs["bq"]) or np.any(inputs["bkv"]) or np.any(inputs["bo"])
        or np.any(inputs["beta_s"]) or np.any(inputs["beta_c"]))
    if has_bias not in _NC_CACHE:
        nc = build_bass(has_bias)
        if not nc.is_finalized():
            nc.finalize()
        _NC_CACHE[has_bias] = nc
    nc = _NC_CACHE[has_bias]
    in_maps = make_in_maps(**inputs)
    res = run_bass_kernel_spmd(nc, in_maps, core_ids=list(range(N_CORES)))
    LAST_RESULT = res
    outs = res.results
    full = np.empty((B, N_SINK, DIM), np.float32)
    for b in range(B):
        full[b] = outs[GROUP * b]["out"]
    return full



# revision 57
# speedup vs baseline: 1.0336x; 1.0336x over previous
"""Trainium2 distributed kernel for nn_AttentionLayer (dense cross-attention
with sink-competition softmax) — v2.

Sharding: 8 cores = 2 batches x 4 head-pairs.  Core c handles batch c//4 and
heads {2*(c%4), 2*(c%4)+1}.

v2 structural changes vs v1:
  - Host ships activations pre-transposed ([feat, tokens], bf16, token-tile
    blocked) so the kernel does zero PE transposes of activations and half
    the HBM traffic.
  - LayerNorm folded into the projections:  LN(x)@W = rstd*(x@W' - c X mu) + b
    with c = gamma@W.  Sums/sums-of-squares come from ones-matmuls against
    the transposed activations (sliding-ones lhsT accumulates all token
    tiles into one [10, 512] PSUM block); squares computed on the otherwise
    idle GpSimd engine; rstd = exp(-0.5*ln(var+eps)) keeps ScalarE in the
    ln/exp table sets only.
  - Key-side rstd is folded into the Exp's per-partition scale AP, query-side
    rstd applied via a rank-1 broadcast, so normalized q/k are never
    materialized separately.
  - sim matmuls for the two heads are row-packed (K=64 at array rows 0-63 /
    64-127) so they run concurrently in the PE array.
  - Output collective is an AllGather of the per-core [128, 1024] bf16
    attention output (instead of ReduceScatter of [1024, 512] f32); each
    core then computes its 256 output rows against the full Wo.
"""

import contextlib
import os
import sys

for _p in ("/opt/trn_rl_repo", "/root/.axon_site/_ro/trn_rl_repo"):
    if os.path.isdir(_p) and _p not in sys.path:
        sys.path.insert(0, _p)

import numpy as np
import ml_dtypes

# Defensive: concourse.bass_utils imports antenv.axon_hooks on the traced
# path; provide a no-op registry if the environment lacks it so tracing
# degrades instead of crashing.
try:
    import antenv.axon_hooks  # noqa: F401
except Exception:
    import types as _types

    _ah = _types.ModuleType("antenv.axon_hooks")
    _ah._hook = None
    _ah.set_axon_ntff_profile_hook = lambda h: setattr(_ah, "_hook", h)
    _ah.get_axon_ntff_profile_hook = lambda: getattr(_ah, "_hook", None)
    try:
        import antenv as _antenv
        _antenv.axon_hooks = _ah
    except Exception:
        pass
    sys.modules["antenv.axon_hooks"] = _ah

import concourse.bass as bass
import concourse.bacc as bacc
import concourse.mybir as mybir
import concourse.tile as tile
from concourse.bass_utils import run_bass_kernel_spmd

# Pin Exp/Ln (the only table-backed activations we use) to the one set that
# contains both, so the table loader never thrashes between the exp-only and
# ln-only sets (each switch costs ~2.7us of ScalarE time).
_ORIG_GAT = bacc.get_activation_tables


def _gat_single_set(arch):
    tabs = dict(_ORIG_GAT(arch))
    _AF = mybir.ActivationFunctionType
    pinned = {_AF.Exp, _AF.Ln, _AF.Square}
    if any(pinned <= fns for fns in tabs.values()):
        out = {}
        seen = False
        for name, fns in tabs.items():
            if not seen and pinned <= fns:
                out[name] = fns
                seen = True
            else:
                out[name] = fns - pinned
        return out
    return tabs


bacc.get_activation_tables = _gat_single_set

F32 = mybir.dt.float32
BF16 = mybir.dt.bfloat16
FP8 = mybir.dt.float8e4
DR = mybir.MatmulPerfMode.DoubleRow
AF = mybir.ActivationFunctionType
ALU = mybir.AluOpType
S_LT = 4096.0  # scale lifting lt = v/Z into fp8e4 range (cancels in av/C)

B, N_SINK, N_SRC, DIM, HID, H = 2, 1024, 4096, 512, 512, 8
D_HEAD = HID // H            # 64
EPS = 1e-6
SCALE = D_HEAD ** -0.5       # 0.125
N_CORES = 8
GROUP = 4                    # cores per batch group
N_TT = 10                    # token tiles of 512 (2 sink + 8 source)
N_KT = 32                    # key tiles of 128
OUT_ROWS = N_SINK // GROUP   # 256 output rows per core

LAST_RESULT = None


def _stat_row(jt):
    """token-tile index for source key-tile jt (tokens 128*jt..+128)."""
    return 2 + jt // 4


def _col_idx(jt):
    """column in the rstd-cols tiles for key-tile jt (8-row source block)."""
    return 8 * (jt % 4) + jt // 4


def build_bass(has_bias=True):
    nc = bacc.Bacc(None, target_bir_lowering=False, debug=False,
                   num_devices=N_CORES)

    for val in (EPS, SCALE, -0.5, 0.5):
        t = nc.alloc_sbuf_tensor(f"const-f32-{val}", [128, 1], F32)
        nc.gpsimd.memset(t.ap(), val)
        nc.const_aps.aps[(F32, val)] = t.ap()
    nc.all_engine_barrier()

    # ---- per-core DRAM parameters ----
    sinkT_d = nc.declare_dram_parameter("sinkT", [128, 2 * 2048], BF16, isOutput=False)
    srcT_d = nc.declare_dram_parameter("srcT", [128, 8 * 2048], BF16, isOutput=False)
    wq_d = nc.declare_dram_parameter("wq", [128, 512], BF16, isOutput=False)
    wk_d = nc.declare_dram_parameter("wk", [128, 512], BF16, isOutput=False)
    wv_d = nc.declare_dram_parameter("wv", [128, 512], BF16, isOutput=False)
    wo_d = nc.declare_dram_parameter("wo", [128, 2048], BF16, isOutput=False)
    r1q_d = nc.declare_dram_parameter("r1q", [1, 128], BF16, isOutput=False)
    r1k_d = nc.declare_dram_parameter("r1k", [1, 128], BF16, isOutput=False)
    r1v_d = nc.declare_dram_parameter("r1v", [1, 128], BF16, isOutput=False)
    bo4_d = nc.declare_dram_parameter("bo4", [1, 512], BF16, isOutput=False)
    ident_d = nc.declare_dram_parameter("ident", [128, 128], BF16, isOutput=False)
    slide_d = nc.declare_dram_parameter("slide", [128, 32], BF16, isOutput=False)
    onec_d = nc.declare_dram_parameter("ones_c", [128, 1], BF16, isOutput=False)
    oner_d = nc.declare_dram_parameter("ones_r", [1, 512], BF16, isOutput=False)
    bo_d = nc.declare_dram_parameter("bo_r", [1, 512], BF16, isOutput=False)
    out_d = nc.declare_dram_parameter("out", [N_SINK, DIM], F32, isOutput=True)

    # collective bounce buffers: per-head 4-rank (batch-group) AllGather of
    # the per-core attn-out half [64 hid, 1024 q].  Output rows 64s = group
    # rank s's head-h dims; every core then computes the full batch output
    # and the host keeps one copy per batch.  Head 0's gather overlaps head
    # 1's key loop.
    ag_in = [nc.dram_tensor(f"ag_in{h}", [64, 1024], BF16) for h in range(2)]
    ag_out = [nc.dram_tensor(f"ag_out{h}", [256, 1024], BF16)
              for h in range(2)]
    # bounce row for the 1/C broadcast (DRAM source legally broadcasts)
    rc_d = [nc.dram_tensor(f"rc_d{h}", [1, 1024], BF16) for h in range(2)]
    # bounce rows for the sink-rstd broadcast onto qT
    rq_d = [nc.dram_tensor(f"rq_d{g}", [1, 512], BF16) for g in range(2)]
    # dummy collective to absorb the ~11us first-collective wakeup latency
    warm_in = nc.dram_tensor("warm_in", [1, 64], BF16)
    warm_out = nc.dram_tensor("warm_out", [4, 64], BF16)
    # stats block -> flat-row bounce buffers ([10, 512] -> [1, 5120]),
    # needed only for the rank-1 bias matmuls
    flat_names = ("ir",) if has_bias else ()
    flat_d = {nm: nc.dram_tensor(f"{nm}_flat_d", [1, 10 * 512], BF16)
              for nm in flat_names}

    with tile.TileContext(nc) as tc:
        with tc.tile_pool(name="const", bufs=1) as cp:
            wq = cp.tile([128, 512], BF16, name="wq_sb")
            wk = cp.tile([128, 512], BF16, name="wk_sb")
            wv = cp.tile([128, 512], BF16, name="wv_sb")
            wo = cp.tile([128, 2048], BF16, name="wo_sb")
            r1q = cp.tile([1, 128], BF16, name="r1q_sb")
            r1k = cp.tile([1, 128], BF16, name="r1k_sb")
            r1v = cp.tile([1, 128], BF16, name="r1v_sb")
            bo4 = cp.tile([1, 512], BF16, name="bo4_sb")
            ident = cp.tile([128, 128], BF16, name="id_sb")
            slide = cp.tile([128, 32], BF16, name="slide_sb")
            ones_c = cp.tile([128, 1], BF16, name="ones_c_sb")
            ones_r = cp.tile([1, 512], BF16, name="ones_r_sb")
            bo_r = cp.tile([1, 512], BF16, name="bo_r_sb")
            # activations, blocked [128, 2048*t + 512*cc + tau]; issue the
            # tensors the stats pipeline needs first.
            xT = cp.tile([128, 20 * 1024], BF16, name="xT_sb")
            # Two HWDGE rings, few big transfers each (per-queue DMAs
            # serialize with multi-us gaps, so batch size beats batch count):
            # scalar carries slide + source tiles 2-5 + weights, sync carries
            # sink + source tiles 6-9.
            nc.scalar.dma_start(out=slide[:, :], in_=slide_d[:, :])
            nc.sync.dma_start(out=xT[:, 0:4096], in_=sinkT_d[:, :])
            nc.scalar.dma_start(out=xT[:, 4096:12288],
                                in_=srcT_d[:, 0:8192])
            nc.sync.dma_start(out=xT[:, 12288:20480],
                              in_=srcT_d[:, 8192:16384])
            for sb, dr in ((wq, wq_d), (wk, wk_d), (wv, wv_d),
                           (ident, ident_d), (wo, wo_d), (r1q, r1q_d),
                           (r1k, r1k_d), (r1v, r1v_d), (bo4, bo4_d),
                           (ones_c, onec_d), (ones_r, oner_d),
                           (bo_r, bo_d)):
                nc.scalar.dma_start(out=sb[:, :], in_=dr[:, :])

            # persistent activations / stats
            qT = cp.tile([128, 1024], BF16, name="qT_sb")
            kT = cp.tile([128, 4096], BF16, name="kT_sb")
            v_sb = cp.tile([128, 130 * N_KT], BF16, name="v_sb")
            ir_bf = cp.tile([10, 512], BF16, name="ir_bf")
            rstd_s = cp.tile([2, 512], BF16, name="rstd_s")
            rstd_c = cp.tile([8, 512], BF16, name="rstd_c")
            # flat-row copy (base partition 0) for rank-1 bias operands
            ir_fl = cp.tile([1, 5120], BF16, name="ir_flat")
            scol = cp.tile([128, 40], F32, name="scol_sb")
            rcol = cp.tile([128, 40], F32, name="rcol_sb")
            aoT = cp.tile([128, 1024], BF16, name="aoT_sb")
            ao_s = cp.tile([128, 4 * 1024], BF16, name="ao_s_sb")

            def xslc(t, cc, off, width):
                return xT[:, 2048 * t + 512 * cc + off:
                          2048 * t + 512 * cc + off + width]

            # ------------- stats (sink block first, then source) -------------
            # v_sb data cols are overwritten with v*rstd*S_LT; the memset
            # value survives only in the C columns (64/129 of each block),
            # which must carry the same S_LT scale so it cancels in av/C.
            nc.gpsimd.memset(v_sb[:, :], S_LT)
            # projection pools outlive both the stats and attention regions
            outer = contextlib.ExitStack()
            pjp = outer.enter_context(
                tc.tile_pool(name="pj_ps", bufs=2, space="PSUM"))
            bsb = outer.enter_context(tc.tile_pool(name="bc_sb", bufs=2))
            with tc.tile_pool(name="xsq", bufs=6) as sqp, \
                 tc.tile_pool(name="st_ps", bufs=1, space="PSUM") as stp, \
                 tc.tile_pool(name="st_sb", bufs=2) as ssb:
                sqs = {}

                def square(t, cc):
                    # keep ScalarE free for the rstd chain + exp stream
                    xs = xslc(t, cc, 0, 512)
                    sq = sqp.tile([128, 512], BF16, tag="sq", bufs=40,
                                  name=f"sq{t}_{cc}")
                    if (4 * t + cc) % 4 < 3:
                        nc.vector.tensor_tensor(sq[:, :], xs, xs, ALU.mult)
                    else:
                        nc.gpsimd.tensor_tensor(sq[:, :], xs, xs, ALU.mult)
                    sqs[(t, cc)] = sq

                def stat_block(t0, rows, rstd_out, irows):
                    """sums + rstd for token tiles t0..t0+rows-1."""
                    sx = stp.tile([rows, 512], F32, tag=f"sxa{t0}", bufs=1,
                                  name=f"sx_{t0}")
                    sx2 = stp.tile([rows, 512], F32, tag=f"sxb{t0}", bufs=1,
                                   name=f"sx2_{t0}")
                    for t in range(t0, t0 + rows):
                        lh = slide[:, 10 - (t - t0):10 - (t - t0) + rows]
                        for cc in range(4):
                            nc.tensor.matmul(sx[0:rows, :], lh,
                                             xslc(t, cc, 0, 512),
                                             start=(t == t0 and cc == 0),
                                             stop=(t == t0 + rows - 1
                                                   and cc == 3))
                    for t in range(t0, t0 + rows):
                        lh = slide[:, 10 - (t - t0):10 - (t - t0) + rows]
                        for cc in range(4):
                            nc.tensor.matmul(sx2[0:rows, :], lh,
                                             sqs[(t, cc)][:, :],
                                             start=(t == t0 and cc == 0),
                                             stop=(t == t0 + rows - 1
                                                   and cc == 3))
                    mu = ssb.tile([rows, 512], F32, tag=f"mu{t0}", bufs=1,
                                  name=f"mu_{t0}")
                    var = ssb.tile([rows, 512], F32, tag=f"va{t0}", bufs=1,
                                   name=f"var_{t0}")
                    lnv = ssb.tile([rows, 512], F32, tag=f"ln{t0}", bufs=1,
                                   name=f"lnv_{t0}")
                    nc.vector.tensor_scalar(mu[:, :], sx[:, :], 1.0 / DIM,
                                            None, ALU.mult)
                    nc.vector.tensor_scalar(var[:, :], sx2[:, :], 1.0 / DIM,
                                            None, ALU.mult)
                    nc.vector.tensor_tensor(mu[:, :], mu[:, :], mu[:, :],
                                            ALU.mult)
                    nc.vector.tensor_tensor(var[:, :], var[:, :], mu[:, :],
                                            ALU.subtract)
                    nc.scalar.activation(lnv[:, :], var[:, :], AF.Ln,
                                         bias=EPS)
                    nc.scalar.activation(rstd_out[:, :], lnv[:, :], AF.Exp,
                                         scale=-0.5)
                    if has_bias:
                        nc.scalar.activation(ir_bf[irows, :], lnv[:, :],
                                             AF.Exp, scale=0.5)

                def q_proj():
                    for g in range(2):
                        bcs = bsb.tile([128, 512], BF16, tag="bcs",
                                       name=f"bcs{g}")
                        nc.sync.dma_start(out=rq_d[g].ap(),
                                          in_=rstd_s[g:g + 1, :])
                        nc.sync.dma_start(
                            out=bcs[:, :],
                            in_=rq_d[g].ap().partition_broadcast(128))
                        pj = pjp.tile([128, 512], F32, tag="pj",
                                      name=f"pjq{g}")
                        for cc in range(4):
                            nc.tensor.matmul(pj[:, :],
                                             wq[:, 128 * cc:128 * (cc + 1)],
                                             xslc(g, cc, 0, 512),
                                             start=(cc == 0),
                                             stop=(cc == 3 and not has_bias))
                        if has_bias:
                            nc.tensor.matmul(pj[:, :], r1q[0:1, :],
                                             ir_fl[0:1, 512 * g:512 * (g + 1)],
                                             start=False, stop=True)
                        nc.vector.tensor_tensor(qT[:, 512 * g:512 * (g + 1)],
                                                pj[:, :], bcs[:, :], ALU.mult)

                def k_proj(g):
                    t = 2 + g
                    pj = pjp.tile([128, 512], F32, tag="pj", name=f"pjk{g}")
                    for cc in range(4):
                        nc.tensor.matmul(pj[:, :],
                                         wk[:, 128 * cc:128 * (cc + 1)],
                                         xslc(t, cc, 0, 512),
                                         start=(cc == 0),
                                         stop=(cc == 3 and not has_bias))
                    if has_bias:
                        nc.tensor.matmul(pj[:, :], r1k[0:1, :],
                                         ir_fl[0:1, 512 * t:512 * (t + 1)],
                                         start=False, stop=True)
                    nc.vector.tensor_copy(kT[:, 512 * g:512 * (g + 1)],
                                          pj[:, :])

                def v_proj(jt):
                    t, w = _stat_row(jt), 128 * (jt % 4)
                    vp = pjp.tile([128, 512], F32, tag="pj", name=f"vp{jt}")
                    for cc in range(4):
                        nc.tensor.matmul(vp[:, 0:128], xslc(t, cc, w, 128),
                                         wv[:, 128 * cc:128 * (cc + 1)],
                                         start=(cc == 0),
                                         stop=(cc == 3 and not has_bias))
                    if has_bias:
                        nc.tensor.matmul(
                            vp[:, 0:128],
                            ir_fl[0:1, 512 * t + w:512 * t + w + 128],
                            r1v[0:1, :], start=False, stop=True)
                    ci = _col_idx(jt)
                    vb = v_sb[:, 130 * jt:130 * jt + 130]
                    nc.vector.tensor_scalar(vb[:, 0:64], vp[:, 0:64],
                                            rcol[:, ci:ci + 1],
                                            None, ALU.mult)
                    nc.vector.tensor_scalar(vb[:, 65:129], vp[:, 64:128],
                                            rcol[:, ci:ci + 1],
                                            None, ALU.mult)

                for t in range(2):
                    for cc in range(4):
                        square(t, cc)
                stat_block(0, 2, rstd_s, slice(0, 2))
                # the first projection band runs on the PE while the source
                # tiles are still arriving
                q_proj()
                k_proj(0)
                for t in range(2, N_TT):
                    for cc in range(4):
                        square(t, cc)
                stat_block(2, 8, rstd_c, slice(2, 10))
                if has_bias:
                    dview = flat_d["ir"].ap().rearrange(
                        "a (t n) -> (a t) n", t=10)
                    nc.sync.dma_start(out=dview, in_=ir_bf[:, :])
                    nc.sync.dma_start(out=ir_fl[:, :],
                                      in_=flat_d["ir"].ap())

                # rstd columns for the source tiles (exp scale + v scaling)
                with tc.tile_pool(name="tp_ps", bufs=2, space="PSUM") as tpp:
                    for c4 in range(4):
                        tp = tpp.tile([128, 8], BF16, tag="tp", name=f"tp{c4}")
                        nc.tensor.transpose(
                            tp[:, :], rstd_c[0:8, 128 * c4:128 * (c4 + 1)],
                            ident[0:8, 0:8])
                        nc.vector.tensor_scalar(
                            scol[:, 8 * c4:8 * (c4 + 1)], tp[:, :], SCALE,
                            None, ALU.mult)
                        nc.vector.tensor_scalar(
                            rcol[:, 8 * c4:8 * (c4 + 1)], tp[:, :], S_LT,
                            None, ALU.mult)
                # first v band, now that rcol exists
                for j in range(4):
                    v_proj(j)

            # ------------- head-split attention, projections interleaved -------------
            with tc.tile_pool(name="acc_ps", bufs=1, space="PSUM") as accp, \
                 tc.tile_pool(name="sim_ps", bufs=2, space="PSUM") as simp, \
                 tc.tile_pool(name="att", bufs=4) as ap_, \
                 tc.tile_pool(name="rs", bufs=4) as rsp, \
                 tc.tile_pool(name="ep_sb", bufs=2) as epp:

                # warm the CC stream well before AG1 (no input deps; issued
                # here so its SDMA traffic stays clear of the input loads)
                nc.gpsimd.collective_compute(
                    "AllGather", ALU.bypass,
                    replica_groups=[[0, 1, 2, 3], [4, 5, 6, 7]],
                    ins=[warm_in.ap().opt()],
                    outs=[warm_out.ap().opt()],
                )
                for h in range(2):
                    hs = 64 * h
                    acc = accp.tile([65, 1024], F32, tag="acc",
                                    name=f"acc{h}")
                    exs = {}
                    lts = {}

                    def issue_sim(jt, h=h, hs=hs, exs=exs, lts=lts):
                        """sim matmul + exp + lt build for key tile jt."""
                        ci = _col_idx(jt)
                        s2 = rsp.tile([128, 1], F32, tag="s2",
                                      name=f"s2_{h}_{jt}")
                        sim = simp.tile([128, 1024], F32, tag="sim",
                                        name=f"sim{h}_{jt}")
                        for qc in range(2):
                            nc.tensor.matmul(
                                sim[:, 512 * qc:512 * (qc + 1)],
                                kT[hs:hs + 64, 128 * jt:128 * (jt + 1)],
                                qT[hs:hs + 64, 512 * qc:512 * (qc + 1)],
                                start=True, stop=True)
                        ex = ap_.tile([128, 1024], BF16, tag="ex",
                                      bufs=6, name=f"ex{h}_{jt}")
                        nc.scalar.activation(ex[:, :], sim[:, :],
                                             AF.Exp,
                                             scale=scol[:, ci:ci + 1],
                                             accum_out=s2[:, 0:1])
                        exs[jt] = ex
                        rs2 = rsp.tile([128, 1], F32, tag="rs2",
                                       name=f"rs2_{h}_{jt}")
                        nc.vector.reciprocal(rs2[:, :], s2[:, :])
                        lt = ap_.tile([128, 65], BF16, tag="lt", bufs=3,
                                      name=f"lt{h}_{jt}")
                        vb = v_sb[:, 130 * jt + 65 * h:
                                  130 * jt + 65 * (h + 1)]
                        nc.vector.tensor_scalar(lt[:, :], vb,
                                                rs2[:, 0:1], None, ALU.mult)
                        lts[jt] = lt

                    def issue_av(jt, acc=acc, exs=exs, lts=lts):
                        lt = lts.pop(jt)
                        ex = exs.pop(jt)
                        for qc in range(2):
                            nc.tensor.matmul(
                                acc[0:65, 512 * qc:512 * (qc + 1)],
                                lt[:, :],
                                ex[:, 512 * qc:512 * (qc + 1)],
                                start=(jt == 0), stop=(jt == N_KT - 1))

                    if h == 0:
                        # interleave the remaining k/v projections with head
                        # 0's key loop: band g's projections land just before
                        # the sims that consume them (band 0 was issued
                        # during the stats region).
                        for g in range(8):
                            if g > 0:
                                k_proj(g)
                                for j in range(4):
                                    v_proj(4 * g + j)
                            for jt in range(4 * g, 4 * g + 4):
                                issue_sim(jt)
                                if jt > 0:
                                    issue_av(jt - 1)
                        issue_av(N_KT - 1)
                    else:
                        issue_sim(0)
                        for jt in range(N_KT):
                            if jt + 1 < N_KT:
                                issue_sim(jt + 1)
                            issue_av(jt)

                    # head epilogue: 1/C = exp(-ln C), DMA-broadcast to the
                    # 64 row partitions, normalize, ship to the collective.
                    lnC = epp.tile([1, 1024], F32, tag="lnC", name=f"lnC{h}")
                    nc.scalar.activation(lnC[:, :], acc[64:65, :], AF.Ln)
                    rcb = epp.tile([1, 1024], BF16, tag="rcb", name=f"rcb{h}")
                    nc.scalar.activation(rcb[:, :], lnC[:, :], AF.Exp,
                                         scale=-1.0)
                    bcs = epp.tile([64, 1024], BF16, tag="bcs", name=f"bcs{h}")
                    nc.sync.dma_start(out=rc_d[h].ap(), in_=rcb[0:1, :])
                    nc.sync.dma_start(
                        out=bcs[:, :],
                        in_=rc_d[h].ap().partition_broadcast(64))
                    nc.vector.tensor_tensor(aoT[hs:hs + 64, :], acc[0:64, :],
                                            bcs[:, :], ALU.mult)
                    nc.sync.dma_start(out=ag_in[h].ap(),
                                      in_=aoT[hs:hs + 64, :])
                    nc.gpsimd.collective_compute(
                        "AllGather", ALU.bypass,
                        replica_groups=[[0, 1, 2, 3], [4, 5, 6, 7]],
                        ins=[ag_in[h].ap().opt()],
                        outs=[ag_out[h].ap().opt()],
                    )

            outer.close()

            # ---- readback + full-batch final projection ----
            # ao_s partitions 0-63 = even-head dims, 64-127 = odd-head dims;
            # columns are [q-chunk, rank, q].  The even-head half of every
            # final accumulation runs while head 1's AllGather drains.
            for qc in range(8):
                for h in range(2):
                    eng = nc.sync if qc % 2 == 0 else nc.scalar
                    eng.dma_start(
                        out=ao_s[64 * h:64 * (h + 1),
                                 512 * qc:512 * (qc + 1)].rearrange(
                            "p (s n) -> p s n", s=4),
                        in_=ag_out[h].ap()[:, 128 * qc:128 * (qc + 1)]
                        .rearrange("(s p) n -> p s n", s=4))

            with tc.tile_pool(name="f_ps", bufs=8, space="PSUM") as fpp, \
                 tc.tile_pool(name="fout", bufs=4) as fop:
                fs = []
                # even-head half of every accumulation first: it only needs
                # AG1, so it runs while head 1's AllGather drains.
                for qc in range(8):
                    f = fpp.tile([128, 512], F32, tag="f", name=f"f{qc}")
                    fs.append(f)
                    for blk in range(4):
                        nc.tensor.matmul(
                            f[:, :],
                            ao_s[0:64, 512 * qc + 128 * blk:
                                 512 * qc + 128 * (blk + 1)],
                            wo[0:64, 512 * blk:512 * (blk + 1)],
                            start=(blk == 0), stop=False)
                for qc in range(8):
                    f = fs[qc]
                    for blk in range(4):
                        nc.tensor.matmul(
                            f[:, :],
                            ao_s[64:128, 512 * qc + 128 * blk:
                                 512 * qc + 128 * (blk + 1)],
                            wo[64:128, 512 * blk:512 * (blk + 1)],
                            start=False,
                            stop=(blk == 3 and not has_bias))
                    if has_bias:
                        nc.tensor.matmul(f[:, :], ones_r[0:1, 0:128],
                                         bo_r[0:1, :], start=False, stop=True)
                    fo = fop.tile([128, 512], F32, tag="fo", name=f"fo{qc}")
                    nc.vector.tensor_copy(fo[:, :], f[:, :])
                    nc.sync.dma_start(out=out_d[128 * qc:128 * (qc + 1), :],
                                      in_=fo[:, :])

    return nc


def _blk(xT):
    """[512, T] f32 -> [128, 4*T] bf16, col = 2048*t + 512*cc + tau."""
    T = xT.shape[1]
    nt = T // 512
    out = xT.reshape(4, 128, nt, 512).transpose(1, 2, 0, 3).reshape(128, 4 * T)
    return np.ascontiguousarray(out).astype(ml_dtypes.bfloat16)


def _chunked(w_loc):
    """[512, 128] -> [128, 512] with col = 128*cc + d."""
    return np.ascontiguousarray(
        w_loc.reshape(4, 128, 128).transpose(1, 0, 2).reshape(128, 512))


def make_in_maps(sink, source, gamma_s, beta_s, gamma_c, beta_c,
                 Wq, bq, Wkv, bkv, Wo, bo):
    f32 = np.float32
    bf16 = ml_dtypes.bfloat16
    cq = (gamma_s @ Wq).astype(f32)
    ck = (gamma_c @ Wkv[:, :HID]).astype(f32)
    cv = (gamma_c @ Wkv[:, HID:]).astype(f32)
    # LN fold: rstd_i*(x_i @ W_eff) + b_eff == LN(x_i) @ (gamma*W) + b, with
    # the mean correction folded into the weights as a rank-1 update.
    Wq_eff = (Wq * gamma_s[:, None] - cq[None, :] / DIM).astype(f32)
    bq_eff = (bq + beta_s @ Wq).astype(f32)
    Wkv_eff = (Wkv * gamma_c[:, None]
               - np.concatenate([ck, cv])[None, :] / DIM).astype(f32)
    bkv_eff = (bkv + beta_c @ Wkv).astype(f32)
    Wk_f, Wv_f = Wkv_eff[:, :HID], Wkv_eff[:, HID:]
    bk_f, bv_f = bkv_eff[:HID], bkv_eff[HID:]

    ident = np.eye(128, dtype=f32).astype(bf16)
    slide = np.zeros((128, 32), f32)
    slide[:, 10] = 1.0
    slide = slide.astype(bf16)
    ones_c = np.ones((128, 1), f32).astype(bf16)
    ones_r = np.ones((1, 512), f32).astype(bf16)
    bo_r = bo.reshape(1, 512).astype(bf16)
    bo4 = (bo / GROUP).reshape(1, 512).astype(bf16)

    # [128, 4*512]: block s = Wo rows for group-rank s's hid slice.
    wo_grp = np.concatenate(
        [Wo[128 * s:128 * (s + 1), :] for s in range(GROUP)], axis=1)

    in_maps = []
    for c in range(N_CORES):
        b, hp = c // GROUP, c % GROUP
        cols = slice(128 * hp, 128 * hp + 128)
        in_maps.append({
            "sinkT": _blk(np.ascontiguousarray(sink[b].T).astype(f32)),
            "srcT": _blk(np.ascontiguousarray(source[b].T).astype(f32)),
            "wq": _chunked(Wq_eff[:, cols]).astype(bf16),
            "wk": _chunked(Wk_f[:, cols]).astype(bf16),
            "wv": _chunked(Wv_f[:, cols]).astype(bf16),
            "wo": wo_grp.astype(bf16),
            "r1q": bq_eff[cols][None, :].astype(bf16),
            "r1k": bk_f[cols][None, :].astype(bf16),
            "r1v": bv_f[cols][None, :].astype(bf16),
            "bo4": bo4,
            "ident": ident,
            "slide": slide,
            "ones_c": ones_c,
            "ones_r": ones_r,
            "bo_r": bo_r,
        })
    return in_maps


_NC_CACHE = {}


def kernel(**inputs):
    global LAST_RESULT
    has_bias = bool(
        np.any(inputs["bq"]) or np.any(inputs["bkv"]) or np.any(inputs["bo"])
        or np.any(inputs["beta_s"]) or np.any(inputs["beta_c"]))
    if has_bias not in _NC_CACHE:
        nc = build_bass(has_bias)
        if not nc.is_finalized():
            nc.finalize()
        _NC_CACHE[has_bias] = nc
    nc = _NC_CACHE[has_bias]
    in_maps = make_in_maps(**inputs)
    res = run_bass_kernel_spmd(nc, in_maps, core_ids=list(range(N_CORES)))
    LAST_RESULT = res
    outs = res.results
    full = np.empty((B, N_SINK, DIM), np.float32)
    for b in range(B):
        full[b] = outs[GROUP * b]["out"]
    return full

